# revision 26
# baseline (speedup 1.0000x reference)
"""Radix-2 DIF ambiguity surface: fp8e4 DoubleRow matmuls + analytic norm.

- max chi = chi[k=0,f=0] = (sum|s|^2)^2 (Cauchy-Schwarz, equality at zero
  lag), so normalization folds into the |X|^2 squares as an ACT scale
  (alpha per batch, shipped in scols col 16); no on-device max chain.
- R build: windows bf16; per-partition scalar muls run on ACT + Pool
  (negative-step reads verified on HW), STT on DVE, radix-2 combines on
  DVE writing fp8e4 pair-tiles directly.
- DFT matmuls run in fp8e4 DoubleRow perf mode: stationary pairs
  [128,(2),128] (planes 528B apart), moving table pairs [128,(2),512] --
  contraction 256 per pass, halving PE time vs bf16.  fp32 PSUM accum.
- Direct k-blocks cover k in [0,512); row 0 (k=512) is a thin 8-matmul
  block (Rsum real / Rdiff imaginary at k=512, exact in this op order).
- Mirror rows 1..511 = frev(chi[k]): PE J-flip on chi directly (f32r),
  with the f-reversal folded into the PSUM->SBUF copies via negative-step
  PSUM reads on ACT/DVE (HW-verified; DMA APs reject negative steps).
- chi kept f-interleaved so every store is a contiguous-row DMA.
"""

import numpy as np

import bass_rust
import concourse.bass as bass
import concourse.mybir as mybir
import concourse.tile as tile
import concourse.bass_utils as bass_utils

B, N = 16, 1024
NCORES = 8
BPC = B // NCORES
NKB = 4
KHI = 514  # R columns built: kk in [0, 514) (513 used; even count)
DS_LEN = 2176
PLW = 528  # fp8 pair-plane pitch in elements (16B-aligned for DoubleRow)

f32 = mybir.dt.float32
f32r = mybir.dt.float32r
bf16 = mybir.dt.bfloat16
fp8 = mybir.dt.float8e4
ALU = mybir.AluOpType
DR = mybir.MatmulPerfMode.DoubleRow


def _split_excess_waits(nc):
    for f in nc.m.functions:
        for blk in f.blocks:
            insts = list(blk.instructions)
            new_insts = []
            changed = False
            for inst in insts:
                si = inst.sync_info
                waits = list(si.on_wait) if (si is not None and si.on_wait) else []
                keep_n = 0 if isinstance(inst, mybir.InstDrain) else 1
                if len(waits) > keep_n:
                    changed = True
                    extra = waits[: len(waits) - keep_n]
                    keep = waits[len(waits) - keep_n:]
                    for w in extra:
                        nop = mybir.InstNoOp(
                            name=nc.get_next_instruction_name(), ins=[], outs=[]
                        )
                        nop.engine = inst.engine
                        nop.sync_info = bass_rust.SyncInfo(on_wait=[w], on_update=[])
                        new_insts.append(nop)
                    inst.sync_info = bass_rust.SyncInfo(
                        on_wait=keep,
                        on_update=list(si.on_update) if si.on_update else [],
                    )
                new_insts.append(inst)
            if changed:
                blk.instructions = new_insts
    return nc


TABNAMES = ["tec", "tes", "tesn", "toc", "tos", "tosn"]


def build_nc():
    nc = bass.Bass("TRN2", target_bir_lowering=False, debug=False)

    dsr = nc.dram_tensor("dsr", [BPC, DS_LEN], bf16, kind="ExternalInput")
    dsi = nc.dram_tensor("dsi", [BPC, DS_LEN], bf16, kind="ExternalInput")
    dsni = nc.dram_tensor("dsni", [BPC, DS_LEN], bf16, kind="ExternalInput")
    scols = nc.dram_tensor("scols", [BPC, 128, 17], f32, kind="ExternalInput")
    tabs = {
        nm: nc.dram_tensor(nm, [512, 512], fp8, kind="ExternalInput")
        for nm in TABNAMES
    }
    jmat = nc.dram_tensor("jmat", [128, 128], f32r, kind="ExternalInput")
    out = nc.dram_tensor("out", [BPC, N, N], f32, kind="ExternalOutput")

    with tile.TileContext(nc) as tc:
        with (
            tc.tile_pool(name="const", bufs=1) as constp,
            tc.tile_pool(name="tp", bufs=1) as tp,
            tc.tile_pool(name="rp", bufs=1) as rp,
            tc.tile_pool(name="tmp", bufs=2) as tmpp,
            tc.tile_pool(name="u", bufs=2) as up,
            tc.tile_pool(name="sq", bufs=2) as sqp,
            tc.tile_pool(name="chi", bufs=1) as chip,
            tc.tile_pool(name="ms", bufs=2) as msp,
            tc.tile_pool(name="sm", bufs=1) as smp,
            tc.tile_pool(name="ps", bufs=2, space="PSUM") as psp,
        ):
            tJ = constp.tile([128, 128], f32r, tag="jmat")
            nc.gpsimd.dma_start(tJ[:], jmat[:])
            # fp8 table pair-tiles: plane0 = chunk 2P, plane1 = chunk 2P+1
            TT = {}
            for P in range(2):
                for nm in TABNAMES:
                    t = constp.tile([128, 1024], fp8, tag=f"{nm}{P}")
                    TT[(nm, P)] = t

            def load_tab(nm, P, eng):
                for pl in range(2):
                    r0 = 256 * P + 128 * pl
                    eng.dma_start(
                        TT[(nm, P)][:, 512 * pl:512 * (pl + 1)],
                        tabs[nm][r0:r0 + 128, :],
                    )

            def emit_load(b):
                s = {"b": b, "chis": {}}
                scol = smp.tile([128, 17], f32, tag=f"scol{b}")
                nc.gpsimd.dma_start(scol[:], scols[b])
                s["scol"] = scol
                Tsr = tp.tile([128, 1538], bf16, tag=f"tsr{b % 2}")
                Tsi = tp.tile([128, 1538], bf16, tag=f"tsi{b % 2}")
                Tnsi = tp.tile([128, 1538], bf16, tag=f"tnsi{b % 2}")
                nc.sync.dma_start(Tsr[:], bass.AP(dsr, b * DS_LEN + 384, [[1, 128], [1, 1538]]))
                nc.sync.dma_start(Tsi[:], bass.AP(dsi, b * DS_LEN + 384, [[1, 128], [1, 1538]]))
                nc.sync.dma_start(Tnsi[:], bass.AP(dsni, b * DS_LEN + 384, [[1, 128], [1, 1538]]))
                s["T"] = (Tsr, Tsi, Tnsi)
                s["R"] = [None, None]  # per pair P: (rsr, rsi, rdr, rdi) fp8 pair-tiles
                return s

            def emit_rbuild(s, qs, lo, hi):
                # R^T[m, kk] = s[m] * conj(s)[(m-kk)%N]; radix-2 sum/diff of
                # the m and m+512 halves -> fp8 pair-tiles for DoubleRow.
                Tsr, Tsi, Tnsi = s["T"]
                scol = s["scol"]
                bt = s["b"] % 2
                n = hi - lo
                for q in qs:
                    m0 = 128 * q
                    P, pl = q // 2, q % 2
                    terms = []
                    for half, woff in ((0, 1024 + m0), (1, 1536 + m0)):
                        sr_c = scol[:, q + 4 * half:q + 4 * half + 1]
                        si_c = scol[:, 8 + q + 4 * half:9 + q + 4 * half]

                        def w(T):
                            ap = T[:]
                            return bass.AP(ap.tensor, ap.offset + woff - 384 - lo, [ap.ap[0], [-1, n]])

                        w_sr, w_si, w_nsi = w(Tsr), w(Tsi), w(Tnsi)
                        a = tmpp.tile([128, 640], bf16, tag="ta")
                        ur = up.tile([128, 640], bf16, tag=f"ur{half}")
                        # Rr = sr_m*sr_win + si_m*si_win   (conj window)
                        nc.scalar.mul(a[:, 0:n], w_sr, sr_c)
                        nc.vector.scalar_tensor_tensor(
                            ur[:, 0:n], w_si, si_c, a[:, 0:n], op0=ALU.mult, op1=ALU.add
                        )
                        b2 = tmpp.tile([128, 640], bf16, tag="tb")
                        ui = up.tile([128, 640], bf16, tag=f"ui{half}")
                        # Ri = si_m*sr_win - sr_m*si_win
                        if q < 2:
                            nc.scalar.mul(b2[:, 0:n], w_nsi, sr_c)
                        else:
                            nc.vector.tensor_scalar_mul(b2[:, 0:n], w_nsi, sr_c)
                        nc.vector.scalar_tensor_tensor(
                            ui[:, 0:n], w_sr, si_c, b2[:, 0:n], op0=ALU.mult, op1=ALU.add
                        )
                        terms.append((ur, ui))
                    (u1r, u1i), (u2r, u2i) = terms
                    if s["R"][P] is None:
                        rsr = rp.tile([128, 2 * PLW], fp8, tag=f"rsr{P}_{bt}")
                        rsi = rp.tile([128, 2 * PLW], fp8, tag=f"rsi{P}_{bt}")
                        rdr = rp.tile([128, 2 * PLW], fp8, tag=f"rdr{P}_{bt}")
                        rdi = rp.tile([128, 2 * PLW], fp8, tag=f"rdi{P}_{bt}")
                        s["R"][P] = (rsr, rsi, rdr, rdi)
                    rsr, rsi, rdr, rdi = s["R"][P]
                    o = PLW * pl + lo
                    nc.vector.tensor_add(rsr[:, o:o + n], u1r[:, 0:n], u2r[:, 0:n])
                    nc.vector.tensor_sub(rdr[:, o:o + n], u1r[:, 0:n], u2r[:, 0:n])
                    nc.vector.tensor_add(rsi[:, o:o + n], u1i[:, 0:n], u2i[:, 0:n])
                    nc.vector.tensor_sub(rdi[:, o:o + n], u1i[:, 0:n], u2i[:, 0:n])

            def _pair_ap(tile_, c, width):
                ap = tile_[:]
                return bass.AP(ap.tensor, ap.offset + c, [ap.ap[0], [PLW, 2], [1, width]])

            def _tab_ap(nm, P):
                ap = TT[(nm, P)][:]
                return bass.AP(ap.tensor, ap.offset, [ap.ap[0], [512, 2], [1, 512]])

            def emit_sq_add(s, key, xre, xie, xro, xio, rows, addeng=None):
                chi_t = chip.tile([128, N], f32r, tag=f"chi{(5 * s['b'] + (key if isinstance(key, int) else 4)) % 6}")
                ralpha = s["scol"][0:rows, 16:17]
                cap = chi_t[0:rows, :]
                for parity, (xr, xi) in ((0, (xre, xie)), (1, (xro, xio))):
                    sqa = sqp.tile([128, 512], f32, tag="sqa")
                    sqb = sqp.tile([128, 512], f32, tag="sqb")
                    nc.scalar.activation(
                        sqa[0:rows, :], xr[0:rows, :],
                        mybir.ActivationFunctionType.Square, scale=ralpha,
                    )
                    nc.scalar.activation(
                        sqb[0:rows, :], xi[0:rows, :],
                        mybir.ActivationFunctionType.Square, scale=ralpha,
                    )
                    dst = bass.AP(cap.tensor, cap.offset + parity, [cap.ap[0], [2, 512]])
                    eng = addeng if addeng is not None else nc.vector
                    eng.tensor_add(dst, sqa[0:rows, :], sqb[0:rows, :])
                s["chis"][key] = chi_t
                return chi_t

            def emit_kblock(b, s, kb, addeng=None):
                c = 128 * kb
                xre = psp.tile([128, 512], f32, tag="xre")
                xie = psp.tile([128, 512], f32, tag="xie")
                xro = psp.tile([128, 512], f32, tag="xro")
                xio = psp.tile([128, 512], f32, tag="xio")
                for P in range(2):
                    rsr, rsi, rdr, rdi = s["R"][P]
                    first = P == 0
                    last = P == 1
                    psr = _pair_ap(rsr, c, 128)
                    psi = _pair_ap(rsi, c, 128)
                    pdr = _pair_ap(rdr, c, 128)
                    pdi = _pair_ap(rdi, c, 128)
                    nc.tensor.matmul(xre[:], psr, _tab_ap("tec", P), start=first, stop=False, perf_mode=DR)
                    nc.tensor.matmul(xie[:], psi, _tab_ap("tec", P), start=first, stop=False, perf_mode=DR)
                    nc.tensor.matmul(xro[:], pdr, _tab_ap("toc", P), start=first, stop=False, perf_mode=DR)
                    nc.tensor.matmul(xio[:], pdi, _tab_ap("toc", P), start=first, stop=False, perf_mode=DR)
                    nc.tensor.matmul(xre[:], psi, _tab_ap("tes", P), start=False, stop=last, perf_mode=DR)
                    nc.tensor.matmul(xie[:], psr, _tab_ap("tesn", P), start=False, stop=last, perf_mode=DR)
                    nc.tensor.matmul(xro[:], pdi, _tab_ap("tos", P), start=False, stop=last, perf_mode=DR)
                    nc.tensor.matmul(xio[:], pdr, _tab_ap("tosn", P), start=False, stop=last, perf_mode=DR)
                emit_sq_add(s, kb, xre, xie, xro, xio, 128, addeng)

            def emit_thin(b, s, addeng=None):
                # k=512 (out row 0): Rsum real, Rdiff imaginary (exact).
                xre = psp.tile([128, 512], f32, tag="xre")
                xie = psp.tile([128, 512], f32, tag="xie")
                xro = psp.tile([128, 512], f32, tag="xro")
                xio = psp.tile([128, 512], f32, tag="xio")
                for P in range(2):
                    rsr, _, _, rdi = s["R"][P]
                    first = P == 0
                    last = P == 1
                    psr = _pair_ap(rsr, 512, 1)
                    pdi = _pair_ap(rdi, 512, 1)
                    nc.tensor.matmul(xre[0:1, :], psr, _tab_ap("tec", P), start=first, stop=last, perf_mode=DR)
                    nc.tensor.matmul(xie[0:1, :], psr, _tab_ap("tesn", P), start=first, stop=last, perf_mode=DR)
                    nc.tensor.matmul(xro[0:1, :], pdi, _tab_ap("tos", P), start=first, stop=last, perf_mode=DR)
                    nc.tensor.matmul(xio[0:1, :], pdi, _tab_ap("toc", P), start=first, stop=last, perf_mode=DR)
                emit_sq_add(s, "thin", xre, xie, xro, xio, 1, addeng)

            def emit_direct(b, s, kbs, split=False):
                for kb in kbs:
                    chi_t = s["chis"][kb]
                    if kb == "thin":
                        nc.sync.dma_start(
                            bass.AP(out, b * N * N, [[N, 1], [1, N]]),
                            chi_t[0:1, :].bitcast(f32),
                        )
                        continue
                    r0 = 512 + 128 * kb
                    if not split:
                        nc.sync.dma_start(out[b, r0:r0 + 128, :], chi_t[:].bitcast(f32))
                        continue
                    # late stores: split row-halves across both HWDGE queues
                    # for end-of-kernel flush parallelism
                    nc.sync.dma_start(out[b, r0:r0 + 64, :], chi_t[0:64, :].bitcast(f32))
                    nc.scalar.dma_start(out[b, r0 + 64:r0 + 128, :], chi_t[64:128, :].bitcast(f32))

            def emit_mirror(b, s, kbs, split=False):
                # out row 512-k = frev(chi[k]), k in [1,511].  J-flip runs on
                # chi directly (f32r); the f-reversal folds into the
                # PSUM->SBUF copies (negative-step PSUM reads verified on
                # both ACT and DVE).  jy0/jy1 = J @ chi halves:
                #   mj[0]        = jy0[:,0]
                #   mj[1:512]    = jy1[:,511:0:-1]
                #   mj[512]      = jy1[:,0]
                #   mj[513:1024] = jy0[:,511:0:-1]
                for kb in kbs:
                    c = 128 * kb
                    chi_t = s["chis"][kb]
                    npart = 127 if kb == 0 else 128
                    rtop = 511 if kb == 0 else 512 - c
                    mj = msp.tile([128, N], f32, tag=f"mj{kb % 2}")
                    jy0 = psp.tile([128, 512], f32, tag="xre")
                    jy1 = psp.tile([128, 512], f32, tag="xro")
                    nc.tensor.matmul(jy0[:], tJ[:], chi_t[:, 0:512], start=True, stop=True)
                    nc.tensor.matmul(jy1[:], tJ[:], chi_t[:, 512:1024], start=True, stop=True)
                    a0 = jy0[:]
                    a1 = jy1[:]
                    rev1 = bass.AP(a1.tensor, a1.offset + 511, [a1.ap[0], [-1, 511]])
                    rev0 = bass.AP(a0.tensor, a0.offset + 511, [a0.ap[0], [-1, 511]])
                    nc.scalar.copy(mj[:, 0:1], jy0[:, 0:1])
                    nc.scalar.copy(mj[:, 1:512], rev1)
                    nc.vector.tensor_copy(mj[:, 512:513], jy1[:, 0:1])
                    nc.vector.tensor_copy(mj[:, 513:1024], rev0)
                    # after J-flip partition r holds k = c + 127 - r
                    # -> out row 385 - c + r (k=0 at partition 127, dropped
                    # for kb 0)
                    rbot = rtop - npart + 1
                    if not split:
                        nc.sync.dma_start(out[b, rbot:rbot + npart, :], mj[0:npart, :])
                    else:
                        half = npart // 2
                        nc.scalar.dma_start(out[b, rbot:rbot + half, :], mj[0:half, :])
                        nc.sync.dma_start(out[b, rbot + half:rbot + npart, :], mj[half:npart, :])

            # --- pipelined schedule
            s0 = emit_load(0)
            for nm in TABNAMES:
                load_tab(nm, 0, nc.sync)
            emit_rbuild(s0, [0, 1, 2, 3], 0, KHI)
            for nm in TABNAMES:
                load_tab(nm, 1, nc.sync)
            emit_kblock(0, s0, 0, nc.gpsimd)
            emit_kblock(0, s0, 1, nc.gpsimd)
            emit_direct(0, s0, [0])
            emit_mirror(0, s0, [0])
            emit_kblock(0, s0, 2, nc.gpsimd)
            emit_direct(0, s0, [1])
            emit_mirror(0, s0, [1])
            emit_kblock(0, s0, 3, nc.gpsimd)
            s1 = emit_load(1)
            emit_thin(0, s0, nc.gpsimd)
            emit_direct(0, s0, [2])
            emit_rbuild(s1, [0, 1, 2, 3], 0, KHI)
            emit_mirror(0, s0, [2])
            emit_direct(0, s0, [3, "thin"])
            emit_mirror(0, s0, [3])
            emit_kblock(1, s1, 0, nc.gpsimd)
            emit_kblock(1, s1, 1, nc.gpsimd)
            emit_direct(1, s1, [0])
            emit_mirror(1, s1, [0])
            emit_kblock(1, s1, 3, nc.vector)
            emit_direct(1, s1, [1])
            emit_mirror(1, s1, [1])
            emit_kblock(1, s1, 2, nc.vector)
            emit_direct(1, s1, [3], split=True)
            emit_mirror(1, s1, [3], split=True)
            emit_thin(1, s1, nc.vector)
            emit_direct(1, s1, [2], split=True)
            emit_mirror(1, s1, [2], split=True)
            emit_direct(1, s1, ["thin"])

    _split_excess_waits(nc)
    return nc


_NC_CACHE = {}


def _get_nc():
    if "nc" not in _NC_CACHE:
        _NC_CACHE["nc"] = build_nc()
    return _NC_CACHE["nc"]


def _get_tables():
    if "tabs" not in _NC_CACHE:
        import ml_dtypes
        f8 = ml_dtypes.float8_e4m3
        m = np.arange(512, dtype=np.float64)[:, None]
        tp_ = np.arange(512, dtype=np.float64)[None, :]
        t_of = (tp_ + 256) % 512
        ang_e = 2.0 * np.pi * ((m * t_of) % 512) / 512
        ang_o = ang_e + 2.0 * np.pi * m / 1024
        tabs = {
            "tec": np.cos(ang_e).astype(f8),
            "tes": np.sin(ang_e).astype(f8),
            "toc": np.cos(ang_o).astype(f8),
            "tos": np.sin(ang_o).astype(f8),
        }
        tabs["tesn"] = -tabs["tes"]
        tabs["tosn"] = -tabs["tos"]
        _NC_CACHE["tabs"] = (tabs, np.eye(128, dtype=np.float32)[::-1].copy())
    return _NC_CACHE["tabs"]


def make_in_maps(s_real, s_imag):
    import ml_dtypes
    bf = ml_dtypes.bfloat16
    tabs, jnp_ = _get_tables()
    in_maps = []
    for core in range(NCORES):
        sl = slice(core * BPC, (core + 1) * BPC)
        sr = np.asarray(s_real[sl], np.float32)
        si = np.asarray(s_imag[sl], np.float32)
        # analytic normalizer: max chi = (sum |s|^2)^2 at k=f=0.
        # alpha is applied as the ACT scale inside |X|^2.
        ralpha = (
            1.0
            / (sr.astype(np.float64) ** 2 + si.astype(np.float64) ** 2).sum(axis=1)
        ).astype(np.float32)
        dsr = np.tile(sr, (1, 3))[:, :DS_LEN].astype(bf)
        dsi_ = np.tile(si, (1, 3))[:, :DS_LEN].astype(bf)
        dsni = np.tile(-si, (1, 3))[:, :DS_LEN].astype(bf)
        scols = np.empty((BPC, 128, 17), np.float32)
        scols[:, :, 0:8] = sr.reshape(BPC, 8, 128).transpose(0, 2, 1)
        scols[:, :, 8:16] = si.reshape(BPC, 8, 128).transpose(0, 2, 1)
        scols[:, :, 16] = ralpha[:, None]
        im = {"dsr": dsr, "dsi": dsi_, "dsni": dsni, "scols": scols, "jmat": jnp_}
        im.update(tabs)
        in_maps.append(im)
    return in_maps


def kernel(s_real: np.ndarray, s_imag: np.ndarray) -> np.ndarray:
    nc = _get_nc()
    in_maps = make_in_maps(s_real, s_imag)
    res = bass_utils.run_bass_kernel_spmd(nc, in_maps, core_ids=list(range(NCORES)))
    return np.concatenate([np.asarray(r["out"], np.float32) for r in res.results], axis=0)


# revision 28
# speedup vs baseline: 1.1816x; 1.1816x over previous
"""Radix-2 DIF ambiguity surface: fp8e4 DoubleRow matmuls + analytic norm.

- max chi = chi[k=0,f=0] = (sum|s|^2)^2 (Cauchy-Schwarz, equality at zero
  lag), so normalization folds into the |X|^2 squares as an ACT scale
  (alpha per batch, shipped in scols col 16); no on-device max chain.
- R build: windows bf16; per-partition scalar muls run on ACT + Pool
  (negative-step reads verified on HW), STT on DVE, radix-2 combines on
  DVE writing fp8e4 pair-tiles directly.
- DFT matmuls run in fp8e4 DoubleRow perf mode: stationary pairs
  [128,(2),128] (planes 528B apart), moving table pairs [128,(2),512] --
  contraction 256 per pass, halving PE time vs bf16.  fp32 PSUM accum.
- Direct k-blocks cover k in [0,512); row 0 (k=512) is a thin 8-matmul
  block (Rsum real / Rdiff imaginary at k=512, exact in this op order).
- Mirror rows 1..511 = frev(chi[k]): PE J-flip on chi directly (f32r),
  with the f-reversal folded into the PSUM->SBUF copies via negative-step
  PSUM reads on ACT/DVE (HW-verified; DMA APs reject negative steps).
- chi kept f-interleaved so every store is a contiguous-row DMA.
"""

import numpy as np

import bass_rust
import concourse.bass as bass
import concourse.mybir as mybir
import concourse.tile as tile
import concourse.bass_utils as bass_utils

B, N = 16, 1024
NCORES = 8
BPC = B // NCORES
NKB = 4
KHI = 514  # R columns built: kk in [0, 514) (513 used; even count)
DS_LEN = 2176
PLW = 528  # fp8 pair-plane pitch in elements (16B-aligned for DoubleRow)

f32 = mybir.dt.float32
f32r = mybir.dt.float32r
bf16 = mybir.dt.bfloat16
fp8 = mybir.dt.float8e4
ALU = mybir.AluOpType
DR = mybir.MatmulPerfMode.DoubleRow


def _split_excess_waits(nc):
    for f in nc.m.functions:
        for blk in f.blocks:
            insts = list(blk.instructions)
            new_insts = []
            changed = False
            for inst in insts:
                si = inst.sync_info
                waits = list(si.on_wait) if (si is not None and si.on_wait) else []
                keep_n = 0 if isinstance(inst, mybir.InstDrain) else 1
                if len(waits) > keep_n:
                    changed = True
                    extra = waits[: len(waits) - keep_n]
                    keep = waits[len(waits) - keep_n:]
                    for w in extra:
                        nop = mybir.InstNoOp(
                            name=nc.get_next_instruction_name(), ins=[], outs=[]
                        )
                        nop.engine = inst.engine
                        nop.sync_info = bass_rust.SyncInfo(on_wait=[w], on_update=[])
                        new_insts.append(nop)
                    inst.sync_info = bass_rust.SyncInfo(
                        on_wait=keep,
                        on_update=list(si.on_update) if si.on_update else [],
                    )
                new_insts.append(inst)
            if changed:
                blk.instructions = new_insts
    return nc


TABNAMES = ["tec", "tes", "tesn", "toc", "tos", "tosn"]


def build_nc():
    nc = bass.Bass("TRN2", target_bir_lowering=False, debug=False)

    dsr = nc.dram_tensor("dsr", [BPC, DS_LEN], bf16, kind="ExternalInput")
    dsi = nc.dram_tensor("dsi", [BPC, DS_LEN], bf16, kind="ExternalInput")
    dsni = nc.dram_tensor("dsni", [BPC, DS_LEN], bf16, kind="ExternalInput")
    scols = nc.dram_tensor("scols", [BPC, 128, 17], f32, kind="ExternalInput")
    tabs = {
        nm: nc.dram_tensor(nm, [512, 512], fp8, kind="ExternalInput")
        for nm in TABNAMES
    }
    jmat = nc.dram_tensor("jmat", [128, 128], f32r, kind="ExternalInput")
    out = nc.dram_tensor("out", [BPC, N, N], f32, kind="ExternalOutput")

    with tile.TileContext(nc) as tc:
        with (
            tc.tile_pool(name="const", bufs=1) as constp,
            tc.tile_pool(name="tp", bufs=1) as tp,
            tc.tile_pool(name="rp", bufs=1) as rp,
            tc.tile_pool(name="tmp", bufs=2) as tmpp,
            tc.tile_pool(name="u", bufs=2) as up,
            tc.tile_pool(name="sq", bufs=2) as sqp,
            tc.tile_pool(name="chi", bufs=1) as chip,
            tc.tile_pool(name="ms", bufs=2) as msp,
            tc.tile_pool(name="sm", bufs=1) as smp,
            tc.tile_pool(name="ps", bufs=2, space="PSUM") as psp,
        ):
            tJ = constp.tile([128, 128], f32r, tag="jmat")
            nc.gpsimd.dma_start(tJ[:], jmat[:])
            # fp8 table pair-tiles: plane0 = chunk 2P, plane1 = chunk 2P+1
            TT = {}
            for P in range(2):
                for nm in TABNAMES:
                    t = constp.tile([128, 1024], fp8, tag=f"{nm}{P}")
                    TT[(nm, P)] = t

            def load_tab(nm, P, eng):
                for pl in range(2):
                    r0 = 256 * P + 128 * pl
                    eng.dma_start(
                        TT[(nm, P)][:, 512 * pl:512 * (pl + 1)],
                        tabs[nm][r0:r0 + 128, :],
                    )

            def emit_load(b):
                s = {"b": b, "chis": {}}
                scol = smp.tile([128, 17], f32, tag=f"scol{b}")
                nc.gpsimd.dma_start(scol[:], scols[b])
                s["scol"] = scol
                Tsr = tp.tile([128, 1538], bf16, tag=f"tsr{b % 2}")
                Tsi = tp.tile([128, 1538], bf16, tag=f"tsi{b % 2}")
                Tnsi = tp.tile([128, 1538], bf16, tag=f"tnsi{b % 2}")
                nc.sync.dma_start(Tsr[:], bass.AP(dsr, b * DS_LEN + 384, [[1, 128], [1, 1538]]))
                nc.sync.dma_start(Tsi[:], bass.AP(dsi, b * DS_LEN + 384, [[1, 128], [1, 1538]]))
                nc.sync.dma_start(Tnsi[:], bass.AP(dsni, b * DS_LEN + 384, [[1, 128], [1, 1538]]))
                s["T"] = (Tsr, Tsi, Tnsi)
                s["R"] = [None, None]  # per pair P: (rsr, rsi, rdr, rdi) fp8 pair-tiles
                return s

            def emit_rbuild(s, qs, lo, hi):
                # R^T[m, kk] = s[m] * conj(s)[(m-kk)%N]; radix-2 sum/diff of
                # the m and m+512 halves -> fp8 pair-tiles for DoubleRow.
                Tsr, Tsi, Tnsi = s["T"]
                scol = s["scol"]
                bt = s["b"] % 2
                n = hi - lo
                for q in qs:
                    m0 = 128 * q
                    P, pl = q // 2, q % 2
                    terms = []
                    for half, woff in ((0, 1024 + m0), (1, 1536 + m0)):
                        sr_c = scol[:, q + 4 * half:q + 4 * half + 1]
                        si_c = scol[:, 8 + q + 4 * half:9 + q + 4 * half]

                        def w(T):
                            ap = T[:]
                            return bass.AP(ap.tensor, ap.offset + woff - 384 - lo, [ap.ap[0], [-1, n]])

                        w_sr, w_si, w_nsi = w(Tsr), w(Tsi), w(Tnsi)
                        a = tmpp.tile([128, 640], bf16, tag="ta")
                        ur = up.tile([128, 640], bf16, tag=f"ur{half}")
                        # Rr = sr_m*sr_win + si_m*si_win   (conj window)
                        nc.scalar.mul(a[:, 0:n], w_sr, sr_c)
                        nc.vector.scalar_tensor_tensor(
                            ur[:, 0:n], w_si, si_c, a[:, 0:n], op0=ALU.mult, op1=ALU.add
                        )
                        b2 = tmpp.tile([128, 640], bf16, tag="tb")
                        ui = up.tile([128, 640], bf16, tag=f"ui{half}")
                        # Ri = si_m*sr_win - sr_m*si_win
                        nc.vector.tensor_scalar_mul(b2[:, 0:n], w_nsi, sr_c)
                        nc.vector.scalar_tensor_tensor(
                            ui[:, 0:n], w_sr, si_c, b2[:, 0:n], op0=ALU.mult, op1=ALU.add
                        )
                        terms.append((ur, ui))
                    (u1r, u1i), (u2r, u2i) = terms
                    if s["R"][P] is None:
                        rsr = rp.tile([128, 2 * PLW], fp8, tag=f"rsr{P}_{bt}")
                        rsi = rp.tile([128, 2 * PLW], fp8, tag=f"rsi{P}_{bt}")
                        rdr = rp.tile([128, 2 * PLW], fp8, tag=f"rdr{P}_{bt}")
                        rdi = rp.tile([128, 2 * PLW], fp8, tag=f"rdi{P}_{bt}")
                        s["R"][P] = (rsr, rsi, rdr, rdi)
                    rsr, rsi, rdr, rdi = s["R"][P]
                    o = PLW * pl + lo
                    nc.vector.tensor_add(rsr[:, o:o + n], u1r[:, 0:n], u2r[:, 0:n])
                    nc.vector.tensor_sub(rdr[:, o:o + n], u1r[:, 0:n], u2r[:, 0:n])
                    nc.vector.tensor_add(rsi[:, o:o + n], u1i[:, 0:n], u2i[:, 0:n])
                    nc.vector.tensor_sub(rdi[:, o:o + n], u1i[:, 0:n], u2i[:, 0:n])

            def _pair_ap(tile_, c, width):
                ap = tile_[:]
                return bass.AP(ap.tensor, ap.offset + c, [ap.ap[0], [PLW, 2], [1, width]])

            def _tab_ap(nm, P):
                ap = TT[(nm, P)][:]
                return bass.AP(ap.tensor, ap.offset, [ap.ap[0], [512, 2], [1, 512]])

            def emit_sq_add(s, key, xre, xie, xro, xio, rows, addeng=None):
                chi_t = chip.tile([128, N], f32r, tag=f"chi{(5 * s['b'] + (key if isinstance(key, int) else 4)) % 6}")
                ralpha = s["scol"][0:rows, 16:17]
                cap = chi_t[0:rows, :]
                for parity, (xr, xi) in ((0, (xre, xie)), (1, (xro, xio))):
                    sqa = sqp.tile([128, 512], f32, tag="sqa")
                    sqb = sqp.tile([128, 512], f32, tag="sqb")
                    nc.scalar.activation(
                        sqa[0:rows, :], xr[0:rows, :],
                        mybir.ActivationFunctionType.Square, scale=ralpha,
                    )
                    nc.scalar.activation(
                        sqb[0:rows, :], xi[0:rows, :],
                        mybir.ActivationFunctionType.Square, scale=ralpha,
                    )
                    dst = bass.AP(cap.tensor, cap.offset + parity, [cap.ap[0], [2, 512]])
                    eng = addeng if addeng is not None else nc.vector
                    eng.tensor_add(dst, sqa[0:rows, :], sqb[0:rows, :])
                s["chis"][key] = chi_t
                return chi_t

            def emit_kblock(b, s, kb, addeng=None):
                c = 128 * kb
                xre = psp.tile([128, 512], f32, tag="xre")
                xie = psp.tile([128, 512], f32, tag="xie")
                xro = psp.tile([128, 512], f32, tag="xro")
                xio = psp.tile([128, 512], f32, tag="xio")
                for P in range(2):
                    rsr, rsi, rdr, rdi = s["R"][P]
                    first = P == 0
                    last = P == 1
                    psr = _pair_ap(rsr, c, 128)
                    psi = _pair_ap(rsi, c, 128)
                    pdr = _pair_ap(rdr, c, 128)
                    pdi = _pair_ap(rdi, c, 128)
                    nc.tensor.matmul(xre[:], psr, _tab_ap("tec", P), start=first, stop=False, perf_mode=DR)
                    nc.tensor.matmul(xie[:], psi, _tab_ap("tec", P), start=first, stop=False, perf_mode=DR)
                    nc.tensor.matmul(xro[:], pdr, _tab_ap("toc", P), start=first, stop=False, perf_mode=DR)
                    nc.tensor.matmul(xio[:], pdi, _tab_ap("toc", P), start=first, stop=False, perf_mode=DR)
                    nc.tensor.matmul(xre[:], psi, _tab_ap("tes", P), start=False, stop=last, perf_mode=DR)
                    nc.tensor.matmul(xie[:], psr, _tab_ap("tesn", P), start=False, stop=last, perf_mode=DR)
                    nc.tensor.matmul(xro[:], pdi, _tab_ap("tos", P), start=False, stop=last, perf_mode=DR)
                    nc.tensor.matmul(xio[:], pdr, _tab_ap("tosn", P), start=False, stop=last, perf_mode=DR)
                emit_sq_add(s, kb, xre, xie, xro, xio, 128, addeng)

            def emit_thin(b, s, addeng=None):
                # k=512 (out row 0): Rsum real, Rdiff imaginary (exact).
                xre = psp.tile([128, 512], f32, tag="xre")
                xie = psp.tile([128, 512], f32, tag="xie")
                xro = psp.tile([128, 512], f32, tag="xro")
                xio = psp.tile([128, 512], f32, tag="xio")
                for P in range(2):
                    rsr, _, _, rdi = s["R"][P]
                    first = P == 0
                    last = P == 1
                    psr = _pair_ap(rsr, 512, 1)
                    pdi = _pair_ap(rdi, 512, 1)
                    nc.tensor.matmul(xre[0:1, :], psr, _tab_ap("tec", P), start=first, stop=last, perf_mode=DR)
                    nc.tensor.matmul(xie[0:1, :], psr, _tab_ap("tesn", P), start=first, stop=last, perf_mode=DR)
                    nc.tensor.matmul(xro[0:1, :], pdi, _tab_ap("tos", P), start=first, stop=last, perf_mode=DR)
                    nc.tensor.matmul(xio[0:1, :], pdi, _tab_ap("toc", P), start=first, stop=last, perf_mode=DR)
                emit_sq_add(s, "thin", xre, xie, xro, xio, 1, addeng)

            def emit_direct(b, s, kbs):
                for kb in kbs:
                    chi_t = s["chis"][kb]
                    if kb == "thin":
                        nc.sync.dma_start(
                            bass.AP(out, b * N * N, [[N, 1], [1, N]]),
                            chi_t[0:1, :].bitcast(f32),
                        )
                        continue
                    # split row-halves across both HWDGE queues: halves the
                    # per-DMA payload a single hot DMA engine can accumulate
                    r0 = 512 + 128 * kb
                    nc.sync.dma_start(out[b, r0:r0 + 64, :], chi_t[0:64, :].bitcast(f32))
                    nc.scalar.dma_start(out[b, r0 + 64:r0 + 128, :], chi_t[64:128, :].bitcast(f32))

            def emit_mirror(b, s, kbs):
                # out row 512-k = frev(chi[k]), k in [1,511].  J-flip runs on
                # chi directly (f32r); the f-reversal folds into the
                # PSUM->SBUF copies (negative-step PSUM reads verified on
                # both ACT and DVE).  jy0/jy1 = J @ chi halves:
                #   mj[0]        = jy0[:,0]
                #   mj[1:512]    = jy1[:,511:0:-1]
                #   mj[512]      = jy1[:,0]
                #   mj[513:1024] = jy0[:,511:0:-1]
                for kb in kbs:
                    c = 128 * kb
                    chi_t = s["chis"][kb]
                    npart = 127 if kb == 0 else 128
                    rtop = 511 if kb == 0 else 512 - c
                    mj = msp.tile([128, N], f32, tag=f"mj{kb % 2}")
                    jy0 = psp.tile([128, 512], f32, tag="xre")
                    jy1 = psp.tile([128, 512], f32, tag="xro")
                    nc.tensor.matmul(jy0[:], tJ[:], chi_t[:, 0:512], start=True, stop=True)
                    nc.tensor.matmul(jy1[:], tJ[:], chi_t[:, 512:1024], start=True, stop=True)
                    a0 = jy0[:]
                    a1 = jy1[:]
                    rev1 = bass.AP(a1.tensor, a1.offset + 511, [a1.ap[0], [-1, 511]])
                    rev0 = bass.AP(a0.tensor, a0.offset + 511, [a0.ap[0], [-1, 511]])
                    nc.scalar.copy(mj[:, 0:1], jy0[:, 0:1])
                    nc.scalar.copy(mj[:, 1:512], rev1)
                    nc.vector.tensor_copy(mj[:, 512:513], jy1[:, 0:1])
                    nc.vector.tensor_copy(mj[:, 513:1024], rev0)
                    # after J-flip partition r holds k = c + 127 - r
                    # -> out row 385 - c + r (k=0 at partition 127, dropped
                    # for kb 0)
                    rbot = rtop - npart + 1
                    half = npart // 2
                    nc.scalar.dma_start(out[b, rbot:rbot + half, :], mj[0:half, :])
                    nc.sync.dma_start(out[b, rbot + half:rbot + npart, :], mj[half:npart, :])

            # --- pipelined schedule
            s0 = emit_load(0)
            for nm in TABNAMES:
                load_tab(nm, 0, nc.sync)
            emit_rbuild(s0, [0, 1, 2, 3], 0, KHI)
            for nm in TABNAMES:
                load_tab(nm, 1, nc.sync)
            emit_kblock(0, s0, 0, nc.gpsimd)
            emit_kblock(0, s0, 1, nc.gpsimd)
            emit_direct(0, s0, [0])
            emit_mirror(0, s0, [0])
            emit_kblock(0, s0, 2, nc.gpsimd)
            emit_direct(0, s0, [1])
            emit_mirror(0, s0, [1])
            emit_kblock(0, s0, 3, nc.gpsimd)
            s1 = emit_load(1)
            emit_thin(0, s0, nc.gpsimd)
            emit_direct(0, s0, [2])
            emit_rbuild(s1, [0, 1, 2, 3], 0, KHI)
            emit_mirror(0, s0, [2])
            emit_direct(0, s0, [3, "thin"])
            emit_mirror(0, s0, [3])
            emit_kblock(1, s1, 0, nc.gpsimd)
            emit_kblock(1, s1, 1, nc.gpsimd)
            emit_direct(1, s1, [0])
            emit_mirror(1, s1, [0])
            emit_kblock(1, s1, 3, nc.vector)
            emit_direct(1, s1, [1])
            emit_mirror(1, s1, [1])
            emit_kblock(1, s1, 2, nc.vector)
            emit_direct(1, s1, [3])
            emit_mirror(1, s1, [3])
            emit_thin(1, s1, nc.vector)
            emit_direct(1, s1, [2])
            emit_mirror(1, s1, [2])
            emit_direct(1, s1, ["thin"])

    _split_excess_waits(nc)
    return nc


_NC_CACHE = {}


def _get_nc():
    if "nc" not in _NC_CACHE:
        _NC_CACHE["nc"] = build_nc()
    return _NC_CACHE["nc"]


def _get_tables():
    if "tabs" not in _NC_CACHE:
        import ml_dtypes
        f8 = ml_dtypes.float8_e4m3
        m = np.arange(512, dtype=np.float64)[:, None]
        tp_ = np.arange(512, dtype=np.float64)[None, :]
        t_of = (tp_ + 256) % 512
        ang_e = 2.0 * np.pi * ((m * t_of) % 512) / 512
        ang_o = ang_e + 2.0 * np.pi * m / 1024
        tabs = {
            "tec": np.cos(ang_e).astype(f8),
            "tes": np.sin(ang_e).astype(f8),
            "toc": np.cos(ang_o).astype(f8),
            "tos": np.sin(ang_o).astype(f8),
        }
        tabs["tesn"] = -tabs["tes"]
        tabs["tosn"] = -tabs["tos"]
        _NC_CACHE["tabs"] = (tabs, np.eye(128, dtype=np.float32)[::-1].copy())
    return _NC_CACHE["tabs"]


def make_in_maps(s_real, s_imag):
    import ml_dtypes
    bf = ml_dtypes.bfloat16
    tabs, jnp_ = _get_tables()
    in_maps = []
    for core in range(NCORES):
        sl = slice(core * BPC, (core + 1) * BPC)
        sr = np.asarray(s_real[sl], np.float32)
        si = np.asarray(s_imag[sl], np.float32)
        # analytic normalizer: max chi = (sum |s|^2)^2 at k=f=0.
        # alpha is applied as the ACT scale inside |X|^2.
        ralpha = (
            1.0
            / (sr.astype(np.float64) ** 2 + si.astype(np.float64) ** 2).sum(axis=1)
        ).astype(np.float32)
        dsr = np.tile(sr, (1, 3))[:, :DS_LEN].astype(bf)
        dsi_ = np.tile(si, (1, 3))[:, :DS_LEN].astype(bf)
        dsni = np.tile(-si, (1, 3))[:, :DS_LEN].astype(bf)
        scols = np.empty((BPC, 128, 17), np.float32)
        scols[:, :, 0:8] = sr.reshape(BPC, 8, 128).transpose(0, 2, 1)
        scols[:, :, 8:16] = si.reshape(BPC, 8, 128).transpose(0, 2, 1)
        scols[:, :, 16] = ralpha[:, None]
        im = {"dsr": dsr, "dsi": dsi_, "dsni": dsni, "scols": scols, "jmat": jnp_}
        im.update(tabs)
        in_maps.append(im)
    return in_maps


def kernel(s_real: np.ndarray, s_imag: np.ndarray) -> np.ndarray:
    nc = _get_nc()
    in_maps = make_in_maps(s_real, s_imag)
    res = bass_utils.run_bass_kernel_spmd(nc, in_maps, core_ids=list(range(NCORES)))
    return np.concatenate([np.asarray(r["out"], np.float32) for r in res.results], axis=0)
